# revision 5
# baseline (speedup 1.0000x reference)
"""Trainium2 Bass kernel for GQA multi-head attention with RoPE.

Problem: B=2, T=2048, C=2048, 16 q-heads, 4 kv-heads, HD=128, causal, RoPE.

Sharding (8 cores): tensor-parallel over the 4 kv-head groups x data-parallel
over the 2 batch elements. Core c handles batch c//4, kv-group c%4 (4 q-heads).
Each core computes x @ wq/wk/wv for its head group, RoPE, causal attention,
and a partial output projection (rows of wo for its heads). The host sums the
4 partial outputs per batch element.

Numerics: all matmuls run in float32r (TF32-like, full PE speed at free-dim
>= 256). Softmax skips the max-subtraction (scores are bounded ~N(0,1) here),
with the causal mask applied as a -1e5 additive bias on diagonal blocks and
fully-masked tiles skipped entirely.
"""

import sys

sys.path.insert(0, "/opt/trn_rl_repo")

import numpy as np

B, T, C = 2, 2048, 2048
N_KV = 4
G = 4           # q heads per kv head
HD = 128
NCORES = 8
TT = T // 128   # 16 t-tiles
CT = C // 128   # 16 c-tiles
NTC = 4         # 512-wide t chunks
SCALE = float(1.0 / np.sqrt(HD))
MASK_BIAS = -1.0e5

_CACHE = {}
LAST_RESULTS = None


def _build():
    import concourse.tile as tile
    from concourse import mybir, bacc

    f32, f32r = mybir.dt.float32, mybir.dt.float32r

    nc = bacc.Bacc()
    xT = nc.dram_tensor("xT", [C, T], f32r, kind="ExternalInput")
    wqkv = nc.dram_tensor("wqkv", [C, 768], f32r, kind="ExternalInput")
    wo = nc.dram_tensor("wo", [G * HD, C], f32r, kind="ExternalInput")
    fcos = nc.dram_tensor("fcos", [T, 64], f32, kind="ExternalInput")
    fsin = nc.dram_tensor("fsin", [T, 64], f32, kind="ExternalInput")
    cident = nc.dram_tensor("cident", [128, 128], f32r, kind="ExternalInput")
    cones = nc.dram_tensor("cones", [128, 1], f32r, kind="ExternalInput")
    conesrow = nc.dram_tensor("conesrow", [1, 128], f32r, kind="ExternalInput")
    ctri = nc.dram_tensor("ctri", [128, 128], f32, kind="ExternalInput")
    out = nc.dram_tensor("out", [T, C], f32, kind="ExternalOutput")

    with tile.TileContext(nc) as tc:
        with (
            tc.tile_pool(name="weights", bufs=1) as wpool,
            tc.tile_pool(name="consts", bufs=1) as cpool,
            tc.tile_pool(name="persist", bufs=1) as ppool,
            tc.tile_pool(name="outbuf", bufs=3) as opool,
        ):
            # ---- constants / weights / tables ----
            wqkv_sb = wpool.tile([128, CT, 768], f32r, tag="w")
            nc.sync.dma_start(wqkv_sb[:], wqkv[:].rearrange("(n p) m -> p n m", p=128))
            fcos_sb = cpool.tile([128, TT, 64], f32)
            fsin_sb = cpool.tile([128, TT, 64], f32)
            nc.sync.dma_start(fcos_sb[:], fcos[:].rearrange("(n p) j -> p n j", p=128))
            nc.sync.dma_start(fsin_sb[:], fsin[:].rearrange("(n p) j -> p n j", p=128))
            ident_sb = cpool.tile([128, 128], f32r)
            nc.sync.dma_start(ident_sb[:], cident[:])
            ones_sb = cpool.tile([128, 1], f32r)
            nc.sync.dma_start(ones_sb[:], cones[:])
            onesrow_sb = cpool.tile([1, 128], f32r)
            nc.sync.dma_start(onesrow_sb[:], conesrow[:])
            tri_sb = cpool.tile([128, 128], f32)
            nc.sync.dma_start(tri_sb[:], ctri[:])

            # ---- persistent activations ----
            qT_sb = ppool.tile([128, G, T], f32r)      # [d, h, t]
            kT_sb = ppool.tile([128, T], f32r)         # [d, s]
            v_sb = ppool.tile([128, TT, HD], f32r)     # [s%128, s//128, d]
            outT_sb = ppool.tile([128, G, T], f32r)    # [d, h, t]

            # ================= Phase 1: QKV projection + RoPE + transpose ====
            with (
                tc.tile_pool(name="xt", bufs=3) as xtp,
                tc.tile_pool(name="ropet", bufs=8) as rtp,
                tc.tile_pool(name="qr", bufs=2) as qrp,
                tc.tile_pool(name="kr", bufs=2) as krp,
                tc.tile_pool(name="ppq", bufs=2, space="PSUM") as ppq,
                tc.tile_pool(name="ppkv", bufs=2, space="PSUM") as ppkv,
                tc.tile_pool(name="pptr", bufs=2, space="PSUM") as pptr,
            ):
                for ch in range(T // 256):  # 8 chunks of 256 t
                    ctx1 = nc.named_scope(f"p1_ch{ch}"); ctx1.__enter__()
                    psq = [ppq.tile([128, 512], f32, tag="psq", name="psq") for _ in range(2)]
                    pskv = [ppkv.tile([128, 256], f32, tag="pskv", name="pskv") for _ in range(2)]
                    for ct in range(CT):
                        xt = xtp.tile([128, 256], f32r, tag="xt")
                        nc.sync.dma_start(
                            xt[:], xT[ct * 128:(ct + 1) * 128, ch * 256:(ch + 1) * 256]
                        )
                        for t2 in range(2):
                            lhsT = xt[:, t2 * 128:(t2 + 1) * 128]
                            nc.tensor.matmul(
                                psq[t2][:], lhsT, wqkv_sb[:, ct, 0:512],
                                start=(ct == 0), stop=(ct == CT - 1),
                            )
                            nc.tensor.matmul(
                                pskv[t2][:], lhsT, wqkv_sb[:, ct, 512:768],
                                start=(ct == 0), stop=(ct == CT - 1),
                            )
                    for t2 in range(2):
                        tt = ch * 2 + t2
                        qr = qrp.tile([128, 512], f32r, tag="qr")
                        kr = krp.tile([128, 128], f32r, tag="kr")
                        cos_t = fcos_sb[:, tt, :]
                        sin_t = fsin_sb[:, tt, :]
                        for u in range(5):  # 4 q heads then k
                            if u < 4:
                                src = psq[t2][:, u * 128:(u + 1) * 128]
                                dst = qr[:, u * 128:(u + 1) * 128]
                            else:
                                src = pskv[t2][:, 0:128]
                                dst = kr
                            te, to = src[:, 0:64], src[:, 64:128]
                            a1 = rtp.tile([128, 64], f32, tag="rt")
                            a2 = rtp.tile([128, 64], f32, tag="rt")
                            nc.vector.tensor_mul(a1[:], te, cos_t)
                            nc.vector.tensor_mul(a2[:], to, sin_t)
                            nc.vector.tensor_sub(dst[:, 0:64], a1[:], a2[:])
                            b1 = rtp.tile([128, 64], f32, tag="rt")
                            b2 = rtp.tile([128, 64], f32, tag="rt")
                            nc.vector.tensor_mul(b1[:], te, sin_t)
                            nc.vector.tensor_mul(b2[:], to, cos_t)
                            nc.vector.tensor_add(dst[:, 64:128], b1[:], b2[:])
                        nc.scalar.copy(v_sb[:, tt, :], pskv[t2][:, 128:256])
                        for h in range(G):
                            ptr = pptr.tile([128, 128], f32r, tag="tr")
                            nc.tensor.transpose(
                                ptr[:], qr[:, h * 128:(h + 1) * 128], ident_sb[:]
                            )
                            nc.scalar.copy(
                                qT_sb[:, h, tt * 128:(tt + 1) * 128], ptr[:]
                            )
                        ptr = pptr.tile([128, 128], f32r, tag="tr")
                        nc.tensor.transpose(ptr[:], kr[:], ident_sb[:])
                        nc.scalar.copy(kT_sb[:, tt * 128:(tt + 1) * 128], ptr[:])
                    ctx1.__exit__(None, None, None)

            # ================= Phase 2+3: attention + output projection ======
            with (
                tc.tile_pool(name="expt", bufs=1) as expp,
                tc.tile_pool(name="denr", bufs=2) as denp,
                tc.tile_pool(name="bcs", bufs=2) as bcp,
                tc.tile_pool(name="pssc", bufs=2, space="PSUM") as pssc,
                tc.tile_pool(name="psden", bufs=1, space="PSUM") as psden,
                tc.tile_pool(name="psbc", bufs=1, space="PSUM") as psbc,
                tc.tile_pool(name="psav", bufs=2, space="PSUM") as psav,
                tc.tile_pool(name="pswo", bufs=2, space="PSUM") as pswo,
            ):
                for tc_i in range(NTC):
                    t0 = tc_i * 512
                    n_s = 4 * (tc_i + 1)
                    offs = [128 * (si - 4 * tc_i) if si >= 4 * tc_i else 0
                            for si in range(n_s)]
                    # order: full-width tile first so PSUM has_written covers bank
                    order = [4 * tc_i] + [si for si in range(n_s) if si != 4 * tc_i]
                    ctx2 = nc.named_scope(f"attn_tc{tc_i}"); ctx2.__enter__()
                    for h in range(G):
                        expT = expp.tile([128, TT, 512], f32r, tag="expT")
                        for si in range(n_s):
                            off = offs[si]
                            ps = pssc.tile([128, 512], f32, tag="sc")
                            nc.tensor.matmul(
                                ps[:, off:512],
                                kT_sb[:, si * 128:(si + 1) * 128],
                                qT_sb[:, h, t0 + off:t0 + 512],
                                start=True, stop=True,
                            )
                            if si >= 4 * tc_i:
                                nc.vector.tensor_add(
                                    ps[:, off:off + 128], ps[:, off:off + 128],
                                    tri_sb[:],
                                )
                            nc.scalar.activation(
                                expT[:, si, off:512], ps[:, off:512],
                                mybir.ActivationFunctionType.Exp, scale=SCALE,
                            )
                        psd = psden.tile([1, 512], f32, tag="den")
                        for i, si in enumerate(order):
                            off = offs[si]
                            nc.tensor.matmul(
                                psd[:, off:512], ones_sb[:], expT[:, si, off:512],
                                start=(i == 0), stop=(i == n_s - 1),
                            )
                        denr = denp.tile([1, 512], f32r, tag="denr")
                        with nc.allow_low_precision(
                            reason="f32r rounding of softmax denominators"
                        ):
                            nc.vector.reciprocal(denr[:], psd[:])
                        pbc = psbc.tile([128, 512], f32, tag="bc")
                        nc.tensor.matmul(pbc[:], onesrow_sb[:], denr[:],
                                         start=True, stop=True)
                        bcs = bcp.tile([128, 512], f32, tag="bcs")
                        nc.scalar.copy(bcs[:], pbc[:])
                        pso = psav.tile([128, 512], f32, tag="av")
                        for i, si in enumerate(order):
                            off = offs[si]
                            nc.tensor.matmul(
                                pso[:, off:512], v_sb[:, si, :], expT[:, si, off:512],
                                start=(i == 0), stop=(i == n_s - 1),
                            )
                        nc.vector.tensor_mul(
                            outT_sb[:, h, t0:t0 + 512], pso[:], bcs[:]
                        )
                    ctx2.__exit__(None, None, None)
                    ctx3 = nc.named_scope(f"wo_tc{tc_i}"); ctx3.__enter__()
                    # load wo once, before first use (reuses wqkv slot via tag)
                    if tc_i == 0:
                        wo_sb = wpool.tile([128, G, C], f32r, tag="w")
                        nc.sync.dma_start(
                            wo_sb[:], wo[:].rearrange("(h p) m -> p h m", p=128)
                        )
                    # output projection for this 512-t block
                    for t2 in range(4):
                        gt = tc_i * 4 + t2
                        for cc in range(4):
                            psw = pswo.tile([128, 512], f32, tag="wo")
                            for h in range(G):
                                nc.tensor.matmul(
                                    psw[:],
                                    outT_sb[:, h, gt * 128:(gt + 1) * 128],
                                    wo_sb[:, h, cc * 512:(cc + 1) * 512],
                                    start=(h == 0), stop=(h == G - 1),
                                )
                            osb = opool.tile([128, 512], f32, tag="osb")
                            nc.scalar.copy(osb[:], psw[:])
                            nc.sync.dma_start(
                                out[gt * 128:(gt + 1) * 128,
                                    cc * 512:(cc + 1) * 512],
                                osb[:],
                            )
                    ctx3.__exit__(None, None, None)

    nc.finalize()
    return nc


def _prep_host(x, freqs_cos, freqs_sin, wq, wk, wv, wo):
    """Build per-core input maps."""
    x = np.asarray(x, dtype=np.float32)
    freqs_cos = np.asarray(freqs_cos, dtype=np.float32)
    freqs_sin = np.asarray(freqs_sin, dtype=np.float32)
    wq = np.asarray(wq, dtype=np.float32)
    wk = np.asarray(wk, dtype=np.float32)
    wv = np.asarray(wv, dtype=np.float32)
    wo = np.asarray(wo, dtype=np.float32)

    perm = np.concatenate([np.arange(0, HD, 2), np.arange(1, HD, 2)])
    xTs = [np.ascontiguousarray(x[b].T) for b in range(B)]
    cident = np.eye(128, dtype=np.float32)
    cones = np.ones((128, 1), dtype=np.float32)
    conesrow = np.ones((1, 128), dtype=np.float32)
    ds, dt = np.meshgrid(np.arange(128), np.arange(128), indexing="ij")
    ctri = np.where(ds <= dt, 0.0, MASK_BIAS).astype(np.float32)

    in_maps = []
    for c in range(NCORES):
        b, kv = c // 4, c % 4
        cols = []
        for g in range(G):
            h = kv * G + g
            cols.append(wq[:, h * HD:(h + 1) * HD][:, perm])
        cols.append(wk[:, kv * HD:(kv + 1) * HD][:, perm])
        cols.append(wv[:, kv * HD:(kv + 1) * HD])
        wqkv_c = np.ascontiguousarray(np.concatenate(cols, axis=1))
        wo_c = np.ascontiguousarray(wo[kv * G * HD:(kv + 1) * G * HD, :])
        in_maps.append({
            "xT": xTs[b],
            "wqkv": wqkv_c,
            "wo": wo_c,
            "fcos": freqs_cos,
            "fsin": freqs_sin,
            "cident": cident,
            "cones": cones,
            "conesrow": conesrow,
            "ctri": ctri,
        })
    return in_maps


def _install_ntff_hook_shim():
    """bass_utils trace=True needs antenv.axon_hooks, absent in this image.
    Provide it in sys.modules and register the ctypes NTFF hook."""
    import types

    if "antenv.axon_hooks" in sys.modules:
        return
    mod = types.ModuleType("antenv.axon_hooks")
    mod._hook = None
    mod.set_axon_ntff_profile_hook = lambda h: setattr(mod, "_hook", h)
    mod.get_axon_ntff_profile_hook = lambda: mod._hook
    sys.modules["antenv.axon_hooks"] = mod
    try:
        from trn_agent_boot.trn_boot import _ntff_profile_via_ctypes

        mod._hook = _ntff_profile_via_ctypes("/opt/axon/libaxon_pjrt.so")
    except Exception:
        pass


def kernel(x, freqs_cos, freqs_sin, wq, wk, wv, wo, trace=False):
    global LAST_RESULTS
    from concourse.bass_utils import run_bass_kernel_spmd

    if trace:
        _install_ntff_hook_shim()

    if "nc" not in _CACHE:
        _CACHE["nc"] = _build()
    nc = _CACHE["nc"]

    in_maps = _prep_host(x, freqs_cos, freqs_sin, wq, wk, wv, wo)
    res = run_bass_kernel_spmd(nc, in_maps, core_ids=list(range(NCORES)),
                               trace=trace)
    LAST_RESULTS = res
    out = np.zeros((B, T, C), dtype=np.float32)
    for c in range(NCORES):
        out[c // 4] += res.results[c]["out"]
    return out


# revision 9
# speedup vs baseline: 1.3102x; 1.3102x over previous
"""Trainium2 Bass kernel for GQA multi-head attention with RoPE.

Problem: B=2, T=2048, C=2048, 16 q-heads, 4 kv-heads, HD=128, causal, RoPE.

Sharding (8 cores): tensor-parallel over the 4 kv-head groups x data-parallel
over the 2 batch elements. Core c handles batch c//4, kv-group c%4 (4 q-heads).
Each core computes x @ wq/wk/wv for its head group, RoPE, causal attention,
and a partial output projection (rows of wo for its heads). The host sums the
4 partial outputs per batch element.

Numerics: all matmuls run in float32r (TF32-like, full PE speed at free-dim
>= 256). Softmax skips the max-subtraction (scores are bounded ~N(0,1) here),
with the causal mask applied as a -1e5 additive bias on diagonal blocks and
fully-masked tiles skipped entirely.
"""

import sys

sys.path.insert(0, "/opt/trn_rl_repo")

import numpy as np

B, T, C = 2, 2048, 2048
N_KV = 4
G = 4           # q heads per kv head
HD = 128
NCORES = 8
TT = T // 128   # 16 t-tiles
CT = C // 128   # 16 c-tiles
NTC = 4         # 512-wide t chunks
SCALE = float(1.0 / np.sqrt(HD))
MASK_BIAS = -1.0e5

_CACHE = {}
LAST_RESULTS = None


def _patch_ldw_opt():
    """Enable walrus's LDWEIGHTS dedup pass (off by default in this repo)."""
    import concourse.bass_utils as bu

    if getattr(bu, "_ldw_patched", False):
        return
    orig = bu.run_command

    def run_command2(argv, **kw):
        argv = ["--enable-ldw-opt=true" if a == "--enable-ldw-opt=false" else a
                for a in argv]
        return orig(argv, **kw)

    bu.run_command = run_command2
    bu._ldw_patched = True


def _build():
    import concourse.bass as bass
    import concourse.tile as tile
    from concourse import mybir, bacc

    def bcast_mid(ap2d, reps):
        """[128, N] AP -> [128, reps, N] with a stride-0 middle dim."""
        return bass.AP(tensor=ap2d.tensor, offset=ap2d.offset,
                       ap=[list(ap2d.ap[0]), [0, reps], list(ap2d.ap[1])])

    f32, f32r = mybir.dt.float32, mybir.dt.float32r

    nc = bacc.Bacc()
    xT = nc.dram_tensor("xT", [C, T], f32r, kind="ExternalInput")
    wqkv = nc.dram_tensor("wqkv", [C, 768], f32r, kind="ExternalInput")
    wo = nc.dram_tensor("wo", [G * HD, C], f32r, kind="ExternalInput")
    fcos = nc.dram_tensor("fcos", [T, 64], f32, kind="ExternalInput")
    fsin = nc.dram_tensor("fsin", [T, 64], f32, kind="ExternalInput")
    cident = nc.dram_tensor("cident", [128, 128], f32r, kind="ExternalInput")
    cones = nc.dram_tensor("cones", [128, 1], f32r, kind="ExternalInput")
    ctri = nc.dram_tensor("ctri", [128, 128], f32, kind="ExternalInput")
    out = nc.dram_tensor("out", [T, C], f32, kind="ExternalOutput")

    with tile.TileContext(nc) as tc:
        with (
            tc.tile_pool(name="consts", bufs=1) as cpool,
            tc.tile_pool(name="persist", bufs=1) as ppool,
            tc.tile_pool(name="outbuf", bufs=3) as opool,
        ):
            ident_sb = cpool.tile([128, 128], f32r)
            nc.sync.dma_start(ident_sb[:], cident[:])
            ones_sb = cpool.tile([128, 1], f32r)
            nc.sync.dma_start(ones_sb[:], cones[:])
            tri_sb = cpool.tile([128, 128], f32)
            nc.sync.dma_start(tri_sb[:], ctri[:])

            # ---- persistent activations ----
            qT_sb = ppool.tile([128, G, T], f32r)      # [d, h, t]
            kT_sb = ppool.tile([128, T], f32r)         # [d, s]
            v_sb = ppool.tile([128, TT, HD], f32r)     # [s%128, s//128, d]
            outT_sb = ppool.tile([128, G, T], f32r)    # [d, h, t]

            # ================= Phase 1: QKV projection + RoPE + transpose ====
            with (
                tc.tile_pool(name="weights", bufs=1) as wpool,
                tc.tile_pool(name="freqs", bufs=1) as fpool,
                tc.tile_pool(name="xt", bufs=3) as xtp,
                tc.tile_pool(name="ropet", bufs=4) as rtp,
                tc.tile_pool(name="qr", bufs=3) as qrp,
                tc.tile_pool(name="kr", bufs=3) as krp,
                tc.tile_pool(name="ppq", bufs=3, space="PSUM") as ppq,
                tc.tile_pool(name="ppkv", bufs=3, space="PSUM") as ppkv,
                tc.tile_pool(name="pptr", bufs=2, space="PSUM") as pptr,
            ):
                wqkv_sb = wpool.tile([128, CT, 768], f32r)
                nc.sync.dma_start(
                    wqkv_sb[:], wqkv[:].rearrange("(n p) m -> p n m", p=128)
                )
                fcos_sb = fpool.tile([128, TT, 64], f32)
                fsin_sb = fpool.tile([128, TT, 64], f32)
                nc.sync.dma_start(
                    fcos_sb[:], fcos[:].rearrange("(n p) j -> p n j", p=128)
                )
                nc.sync.dma_start(
                    fsin_sb[:], fsin[:].rearrange("(n p) j -> p n j", p=128)
                )

                pending_tr = []
                for ch in range(T // 256):  # 8 chunks of 256 t
                    scope = nc.named_scope(f"p1_ch{ch}")
                    scope.__enter__()
                    psq = [ppq.tile([128, 512], f32, tag="psq", name="psq")
                           for _ in range(2)]
                    pskv = [ppkv.tile([128, 256], f32, tag="pskv", name="pskv")
                            for _ in range(2)]
                    for ct in range(CT):
                        xt = xtp.tile([128, 256], f32r, tag="xt")
                        nc.sync.dma_start(
                            xt[:],
                            xT[ct * 128:(ct + 1) * 128, ch * 256:(ch + 1) * 256],
                        )
                        for t2 in range(2):
                            lhsT = xt[:, t2 * 128:(t2 + 1) * 128]
                            nc.tensor.matmul(
                                psq[t2][:], lhsT, wqkv_sb[:, ct, 0:512],
                                start=(ct == 0), stop=(ct == CT - 1),
                            )
                            nc.tensor.matmul(
                                pskv[t2][:], lhsT, wqkv_sb[:, ct, 512:768],
                                start=(ct == 0), stop=(ct == CT - 1),
                            )
                    # rope (batched over the 4 q heads) — emitted now (DVE)
                    this_tr = []
                    for t2 in range(2):
                        tt = ch * 2 + t2
                        qr = qrp.tile([128, 512], f32r, tag="qr")
                        kr = krp.tile([128, 128], f32r, tag="kr")
                        cosb = bcast_mid(fcos_sb[:, tt, :], 4)
                        sinb = bcast_mid(fsin_sb[:, tt, :], 4)
                        qsrc = psq[t2][:].rearrange(
                            "p (h two j) -> p h two j", h=4, two=2
                        )
                        qdst = qr[:].rearrange(
                            "p (h two j) -> p h two j", h=4, two=2
                        )
                        te4, to4 = qsrc[:, :, 0, :], qsrc[:, :, 1, :]
                        a1 = rtp.tile([128, 4, 64], f32, tag="rt")
                        a2 = rtp.tile([128, 4, 64], f32, tag="rt")
                        nc.vector.tensor_mul(a1[:], te4, cosb)
                        nc.vector.tensor_mul(a2[:], to4, sinb)
                        nc.vector.tensor_sub(qdst[:, :, 0, :], a1[:], a2[:])
                        b1 = rtp.tile([128, 4, 64], f32, tag="rt")
                        b2 = rtp.tile([128, 4, 64], f32, tag="rt")
                        nc.vector.tensor_mul(b1[:], te4, sinb)
                        nc.vector.tensor_mul(b2[:], to4, cosb)
                        nc.vector.tensor_add(qdst[:, :, 1, :], b1[:], b2[:])
                        # K rope
                        kte, kto = pskv[t2][:, 0:64], pskv[t2][:, 64:128]
                        cos1 = fcos_sb[:, tt, :]
                        sin1 = fsin_sb[:, tt, :]
                        c1 = rtp.tile([128, 64], f32, tag="rtk")
                        c2 = rtp.tile([128, 64], f32, tag="rtk")
                        nc.vector.tensor_mul(c1[:], kte, cos1)
                        nc.vector.tensor_mul(c2[:], kto, sin1)
                        nc.vector.tensor_sub(kr[:, 0:64], c1[:], c2[:])
                        d1 = rtp.tile([128, 64], f32, tag="rtk")
                        d2 = rtp.tile([128, 64], f32, tag="rtk")
                        nc.vector.tensor_mul(d1[:], kte, sin1)
                        nc.vector.tensor_mul(d2[:], kto, cos1)
                        nc.vector.tensor_add(kr[:, 64:128], d1[:], d2[:])
                        nc.scalar.copy(v_sb[:, tt, :], pskv[t2][:, 128:256])
                        this_tr.append((tt, qr, kr))
                    # transposes for the PREVIOUS chunk (PE stays dense)
                    for tt, qr, kr in pending_tr:
                        for h in range(G):
                            ptr = pptr.tile([128, 128], f32r, tag="tr",
                                            name="ptr")
                            nc.tensor.transpose(
                                ptr[:], qr[:, h * 128:(h + 1) * 128], ident_sb[:]
                            )
                            nc.scalar.copy(
                                qT_sb[:, h, tt * 128:(tt + 1) * 128], ptr[:]
                            )
                        ptr = pptr.tile([128, 128], f32r, tag="tr", name="ptr")
                        nc.tensor.transpose(ptr[:], kr[:], ident_sb[:])
                        nc.scalar.copy(kT_sb[:, tt * 128:(tt + 1) * 128], ptr[:])
                    pending_tr = this_tr
                    scope.__exit__(None, None, None)
                for tt, qr, kr in pending_tr:
                    for h in range(G):
                        ptr = pptr.tile([128, 128], f32r, tag="tr", name="ptr")
                        nc.tensor.transpose(
                            ptr[:], qr[:, h * 128:(h + 1) * 128], ident_sb[:]
                        )
                        nc.scalar.copy(
                            qT_sb[:, h, tt * 128:(tt + 1) * 128], ptr[:]
                        )
                    ptr = pptr.tile([128, 128], f32r, tag="tr", name="ptr")
                    nc.tensor.transpose(ptr[:], kr[:], ident_sb[:])
                    nc.scalar.copy(kT_sb[:, tt * 128:(tt + 1) * 128], ptr[:])

            # ================= Phase 2+3: attention + output projection ======
            with (
                tc.tile_pool(name="wop", bufs=1) as wop,
                tc.tile_pool(name="expt", bufs=2) as expp,
                tc.tile_pool(name="denb", bufs=2) as denp,
                tc.tile_pool(name="bcb", bufs=2) as bcp,
                tc.tile_pool(name="pssc", bufs=3, space="PSUM") as pssc,
                tc.tile_pool(name="psden", bufs=1, space="PSUM") as psden,
                tc.tile_pool(name="psav", bufs=2, space="PSUM") as psav,
                tc.tile_pool(name="pswo", bufs=2, space="PSUM") as pswo,
            ):
                wo_sb = wop.tile([128, G, C], f32r)
                nc.sync.dma_start(
                    wo_sb[:], wo[:].rearrange("(h p) m -> p h m", p=128)
                )

                for tc_i in range(NTC):
                    scope = nc.named_scope(f"attn_tc{tc_i}")
                    scope.__enter__()
                    t0 = tc_i * 512
                    n_s = 4 * (tc_i + 1)
                    offs = [128 * (si - 4 * tc_i) if si >= 4 * tc_i else 0
                            for si in range(n_s)]
                    order = [4 * tc_i] + [si for si in range(n_s)
                                          if si != 4 * tc_i]
                    expTs = {}

                    def emit_scores(h):
                        expT = expp.tile([128, TT, 512], f32r, tag="expT",
                                         name="expT")
                        expTs[h] = expT
                        for si in range(n_s):
                            off = offs[si]
                            ps = pssc.tile([128, 512], f32, tag="sc", name="ps")
                            nc.tensor.matmul(
                                ps[:, off:512],
                                kT_sb[:, si * 128:(si + 1) * 128],
                                qT_sb[:, h, t0 + off:t0 + 512],
                                start=True, stop=True,
                            )
                            if si >= 4 * tc_i:
                                nc.vector.tensor_add(
                                    ps[:, off:off + 128],
                                    ps[:, off:off + 128], tri_sb[:],
                                )
                            nc.scalar.activation(
                                expT[:, si, off:512], ps[:, off:512],
                                mybir.ActivationFunctionType.Exp, scale=SCALE,
                            )

                    def emit_da(h):
                        expT = expTs.pop(h)
                        psd = psden.tile([1, 512], f32, tag="den", name="psd")
                        for i, si in enumerate(order):
                            off = offs[si]
                            nc.tensor.matmul(
                                psd[:, off:512], ones_sb[:],
                                expT[:, si, off:512],
                                start=(i == 0), stop=(i == n_s - 1),
                            )
                        den_r = denp.tile([1, 512], f32, tag="denr",
                                          name="den_r")
                        nc.vector.reciprocal_approx_fast(den_r[:], psd[:])
                        bc = bcp.tile([128, 512], f32, tag="bc", name="bc")
                        nc.gpsimd.partition_broadcast(bc[:], den_r[:])
                        pso = psav.tile([128, 512], f32, tag="av", name="pso")
                        for i, si in enumerate(order):
                            off = offs[si]
                            nc.tensor.matmul(
                                pso[:, off:512], v_sb[:, si, :],
                                expT[:, si, off:512],
                                start=(i == 0), stop=(i == n_s - 1),
                            )
                        nc.vector.tensor_mul(
                            outT_sb[:, h, t0:t0 + 512], pso[:], bc[:]
                        )

                    # 1-deep head pipelining: sc0 sc1 da0 sc2 da1 sc3 da2 da3
                    emit_scores(0)
                    emit_scores(1)
                    emit_da(0)
                    emit_scores(2)
                    emit_da(1)
                    emit_scores(3)
                    emit_da(2)
                    emit_da(3)
                    scope.__exit__(None, None, None)

                    scope = nc.named_scope(f"wo_tc{tc_i}")
                    scope.__enter__()
                    for t2 in range(4):
                        gt = tc_i * 4 + t2
                        for cc in range(4):
                            psw = pswo.tile([128, 512], f32, tag="wo",
                                            name="psw")
                            for h in range(G):
                                nc.tensor.matmul(
                                    psw[:],
                                    outT_sb[:, h, gt * 128:(gt + 1) * 128],
                                    wo_sb[:, h, cc * 512:(cc + 1) * 512],
                                    start=(h == 0), stop=(h == G - 1),
                                )
                            osb = opool.tile([128, 512], f32, tag="osb",
                                             name="osb")
                            nc.scalar.copy(osb[:], psw[:])
                            nc.sync.dma_start(
                                out[gt * 128:(gt + 1) * 128,
                                    cc * 512:(cc + 1) * 512],
                                osb[:],
                            )
                    scope.__exit__(None, None, None)

    nc.finalize()
    return nc


def _prep_host(x, freqs_cos, freqs_sin, wq, wk, wv, wo):
    """Build per-core input maps."""
    x = np.asarray(x, dtype=np.float32)
    freqs_cos = np.asarray(freqs_cos, dtype=np.float32)
    freqs_sin = np.asarray(freqs_sin, dtype=np.float32)
    wq = np.asarray(wq, dtype=np.float32)
    wk = np.asarray(wk, dtype=np.float32)
    wv = np.asarray(wv, dtype=np.float32)
    wo = np.asarray(wo, dtype=np.float32)

    perm = np.concatenate([np.arange(0, HD, 2), np.arange(1, HD, 2)])
    xTs = [np.ascontiguousarray(x[b].T) for b in range(B)]
    cident = np.eye(128, dtype=np.float32)
    cones = np.ones((128, 1), dtype=np.float32)
    ds, dt = np.meshgrid(np.arange(128), np.arange(128), indexing="ij")
    ctri = np.where(ds <= dt, 0.0, MASK_BIAS).astype(np.float32)

    in_maps = []
    for c in range(NCORES):
        b, kv = c // 4, c % 4
        cols = []
        for g in range(G):
            h = kv * G + g
            cols.append(wq[:, h * HD:(h + 1) * HD][:, perm])
        cols.append(wk[:, kv * HD:(kv + 1) * HD][:, perm])
        cols.append(wv[:, kv * HD:(kv + 1) * HD])
        wqkv_c = np.ascontiguousarray(np.concatenate(cols, axis=1))
        wo_c = np.ascontiguousarray(wo[kv * G * HD:(kv + 1) * G * HD, :])
        in_maps.append({
            "xT": xTs[b],
            "wqkv": wqkv_c,
            "wo": wo_c,
            "fcos": freqs_cos,
            "fsin": freqs_sin,
            "cident": cident,
            "cones": cones,
            "ctri": ctri,
        })
    return in_maps


def _install_ntff_hook_shim():
    """bass_utils trace=True needs antenv.axon_hooks, absent in this image.
    Provide it in sys.modules and register the ctypes NTFF hook."""
    import types

    if "antenv.axon_hooks" in sys.modules:
        return
    mod = types.ModuleType("antenv.axon_hooks")
    mod._hook = None
    mod.set_axon_ntff_profile_hook = lambda h: setattr(mod, "_hook", h)
    mod.get_axon_ntff_profile_hook = lambda: mod._hook
    sys.modules["antenv.axon_hooks"] = mod
    try:
        from trn_agent_boot.trn_boot import _ntff_profile_via_ctypes

        mod._hook = _ntff_profile_via_ctypes("/opt/axon/libaxon_pjrt.so")
    except Exception:
        pass


def kernel(x, freqs_cos, freqs_sin, wq, wk, wv, wo, trace=False):
    global LAST_RESULTS
    import os

    if os.environ.get("LDW_OPT"):
        _patch_ldw_opt()
    from concourse.bass_utils import run_bass_kernel_spmd

    if trace:
        _install_ntff_hook_shim()

    if "nc" not in _CACHE:
        _CACHE["nc"] = _build()
    nc = _CACHE["nc"]

    in_maps = _prep_host(x, freqs_cos, freqs_sin, wq, wk, wv, wo)
    res = run_bass_kernel_spmd(nc, in_maps, core_ids=list(range(NCORES)),
                               trace=trace)
    LAST_RESULTS = res
    out = np.zeros((B, T, C), dtype=np.float32)
    for c in range(NCORES):
        out[c // 4] += res.results[c]["out"]
    return out


# revision 10
# speedup vs baseline: 1.8212x; 1.3900x over previous
"""Trainium2 Bass kernel for GQA multi-head attention with RoPE.

Problem: B=2, T=2048, C=2048, 16 q-heads, 4 kv-heads, HD=128, causal, RoPE.

Sharding (8 cores): tensor-parallel over the 4 kv-head groups x data-parallel
over the 2 batch elements. Core c handles batch c//4, kv-group c%4 (4 q-heads).
Each core computes x @ wq/wk/wv for its head group, RoPE, causal attention,
and a partial output projection (rows of wo for its heads). The host sums the
4 partial outputs per batch element.

Numerics: all matmuls run in float32r (TF32-like, full PE speed at free-dim
>= 256). Softmax skips the max-subtraction (scores are bounded ~N(0,1) here),
with the causal mask applied as a -1e5 additive bias on diagonal blocks and
fully-masked tiles skipped entirely.
"""

import sys

sys.path.insert(0, "/opt/trn_rl_repo")

import numpy as np

B, T, C = 2, 2048, 2048
N_KV = 4
G = 4           # q heads per kv head
HD = 128
NCORES = 8
TT = T // 128   # 16 t-tiles
CT = C // 128   # 16 c-tiles
NTC = 4         # 512-wide t chunks
SCALE = float(1.0 / np.sqrt(HD))
MASK_BIAS = -1.0e5

_CACHE = {}
LAST_RESULTS = None


def _patch_ldw_opt():
    """Enable walrus's LDWEIGHTS dedup pass (off by default in this repo)."""
    import concourse.bass_utils as bu

    if getattr(bu, "_ldw_patched", False):
        return
    orig = bu.run_command

    def run_command2(argv, **kw):
        argv = ["--enable-ldw-opt=true" if a == "--enable-ldw-opt=false" else a
                for a in argv]
        return orig(argv, **kw)

    bu.run_command = run_command2
    bu._ldw_patched = True


def _build():
    import concourse.bass as bass
    import concourse.tile as tile
    from concourse import mybir, bacc

    def bcast_mid(ap2d, reps):
        """[128, N] AP -> [128, reps, N] with a stride-0 middle dim."""
        return bass.AP(tensor=ap2d.tensor, offset=ap2d.offset,
                       ap=[list(ap2d.ap[0]), [0, reps], list(ap2d.ap[1])])

    f32, f32r = mybir.dt.float32, mybir.dt.float32r

    nc = bacc.Bacc()
    xT = nc.dram_tensor("xT", [C, T], f32r, kind="ExternalInput")
    wqkv = nc.dram_tensor("wqkv", [C, 768], f32r, kind="ExternalInput")
    wo = nc.dram_tensor("wo", [G * HD, C], f32r, kind="ExternalInput")
    fcos = nc.dram_tensor("fcos", [T, 64], f32, kind="ExternalInput")
    fsin = nc.dram_tensor("fsin", [T, 64], f32, kind="ExternalInput")
    cident = nc.dram_tensor("cident", [128, 128], f32r, kind="ExternalInput")
    cones = nc.dram_tensor("cones", [128, 1], f32r, kind="ExternalInput")
    ctri = nc.dram_tensor("ctri", [128, 128], f32, kind="ExternalInput")
    out = nc.dram_tensor("out", [T, C], f32, kind="ExternalOutput")

    with tile.TileContext(nc) as tc:
        with (
            tc.tile_pool(name="consts", bufs=1) as cpool,
            tc.tile_pool(name="persist", bufs=1) as ppool,
            tc.tile_pool(name="outbuf", bufs=3) as opool,
        ):
            ident_sb = cpool.tile([128, 128], f32r)
            nc.sync.dma_start(ident_sb[:], cident[:])
            ones_sb = cpool.tile([128, 1], f32r)
            nc.sync.dma_start(ones_sb[:], cones[:])
            tri_sb = cpool.tile([128, 128], f32)
            nc.sync.dma_start(tri_sb[:], ctri[:])

            # ---- persistent activations ----
            qT_sb = ppool.tile([128, G, T], f32r)      # [d, h, t]
            kT_sb = ppool.tile([128, T], f32r)         # [d, s]
            v_sb = ppool.tile([128, TT, HD], f32r)     # [s%128, s//128, d]
            outT_sb = ppool.tile([128, G, T], f32r)    # [d, h, t]

            # ================= Phase 1: QKV projection + RoPE + transpose ====
            with (
                tc.tile_pool(name="weights", bufs=1) as wpool,
                tc.tile_pool(name="freqs", bufs=1) as fpool,
                tc.tile_pool(name="xt", bufs=3) as xtp,
                tc.tile_pool(name="ropet", bufs=4) as rtp,
                tc.tile_pool(name="qr", bufs=3) as qrp,
                tc.tile_pool(name="kr", bufs=3) as krp,
                tc.tile_pool(name="ppq", bufs=3, space="PSUM") as ppq,
                tc.tile_pool(name="ppkv", bufs=3, space="PSUM") as ppkv,
                tc.tile_pool(name="pptr", bufs=2, space="PSUM") as pptr,
            ):
                wqkv_sb = wpool.tile([128, CT, 768], f32r)
                nc.sync.dma_start(
                    wqkv_sb[:], wqkv[:].rearrange("(n p) m -> p n m", p=128)
                )
                fcos_sb = fpool.tile([128, TT, 64], f32)
                fsin_sb = fpool.tile([128, TT, 64], f32)
                nc.sync.dma_start(
                    fcos_sb[:], fcos[:].rearrange("(n p) j -> p n j", p=128)
                )
                nc.sync.dma_start(
                    fsin_sb[:], fsin[:].rearrange("(n p) j -> p n j", p=128)
                )

                pending_tr = []
                for ch in range(T // 256):  # 8 chunks of 256 t
                    scope = nc.named_scope(f"p1_ch{ch}")
                    scope.__enter__()
                    psq = [ppq.tile([128, 512], f32, tag="psq", name="psq")
                           for _ in range(2)]
                    pskv = [ppkv.tile([128, 256], f32, tag="pskv", name="pskv")
                            for _ in range(2)]
                    for cg in range(CT // 4):
                        xt = xtp.tile([128, 4, 256], f32r, tag="xt")
                        nc.sync.dma_start(
                            xt[:],
                            xT[:].rearrange("(n p) t -> p n t", p=128)[
                                cg * 4 * 128:cg * 4 * 128 + 128, :, :
                            ] if False else
                            bass.AP(
                                tensor=xT[:].tensor,
                                offset=(cg * 4 * 128) * T + ch * 256,
                                ap=[[T, 128], [128 * T, 4], [1, 256]],
                            ),
                        )
                        for ci in range(4):
                            ct = cg * 4 + ci
                            for t2 in range(2):
                                lhsT = xt[:, ci, t2 * 128:(t2 + 1) * 128]
                                nc.tensor.matmul(
                                    psq[t2][:], lhsT, wqkv_sb[:, ct, 0:512],
                                    start=(ct == 0), stop=(ct == CT - 1),
                                )
                                nc.tensor.matmul(
                                    pskv[t2][:], lhsT, wqkv_sb[:, ct, 512:768],
                                    start=(ct == 0), stop=(ct == CT - 1),
                                )
                    # rope (batched over the 4 q heads) — emitted now (DVE)
                    this_tr = []
                    for t2 in range(2):
                        tt = ch * 2 + t2
                        qr = qrp.tile([128, 512], f32r, tag="qr")
                        kr = krp.tile([128, 128], f32r, tag="kr")
                        cosb = bcast_mid(fcos_sb[:, tt, :], 4)
                        sinb = bcast_mid(fsin_sb[:, tt, :], 4)
                        qsrc = psq[t2][:].rearrange(
                            "p (h two j) -> p h two j", h=4, two=2
                        )
                        qdst = qr[:].rearrange(
                            "p (h two j) -> p h two j", h=4, two=2
                        )
                        te4, to4 = qsrc[:, :, 0, :], qsrc[:, :, 1, :]
                        a1 = rtp.tile([128, 4, 64], f32, tag="rt")
                        a2 = rtp.tile([128, 4, 64], f32, tag="rt")
                        nc.vector.tensor_mul(a1[:], te4, cosb)
                        nc.vector.tensor_mul(a2[:], to4, sinb)
                        nc.vector.tensor_sub(qdst[:, :, 0, :], a1[:], a2[:])
                        b1 = rtp.tile([128, 4, 64], f32, tag="rt")
                        b2 = rtp.tile([128, 4, 64], f32, tag="rt")
                        nc.vector.tensor_mul(b1[:], te4, sinb)
                        nc.vector.tensor_mul(b2[:], to4, cosb)
                        nc.vector.tensor_add(qdst[:, :, 1, :], b1[:], b2[:])
                        # K rope
                        kte, kto = pskv[t2][:, 0:64], pskv[t2][:, 64:128]
                        cos1 = fcos_sb[:, tt, :]
                        sin1 = fsin_sb[:, tt, :]
                        c1 = rtp.tile([128, 64], f32, tag="rtk")
                        c2 = rtp.tile([128, 64], f32, tag="rtk")
                        nc.vector.tensor_mul(c1[:], kte, cos1)
                        nc.vector.tensor_mul(c2[:], kto, sin1)
                        nc.vector.tensor_sub(kr[:, 0:64], c1[:], c2[:])
                        d1 = rtp.tile([128, 64], f32, tag="rtk")
                        d2 = rtp.tile([128, 64], f32, tag="rtk")
                        nc.vector.tensor_mul(d1[:], kte, sin1)
                        nc.vector.tensor_mul(d2[:], kto, cos1)
                        nc.vector.tensor_add(kr[:, 64:128], d1[:], d2[:])
                        nc.scalar.copy(v_sb[:, tt, :], pskv[t2][:, 128:256])
                        this_tr.append((tt, qr, kr))
                    # transposes for the PREVIOUS chunk (PE stays dense)
                    for tt, qr, kr in pending_tr:
                        for h in range(G):
                            ptr = pptr.tile([128, 128], f32r, tag="tr",
                                            name="ptr")
                            nc.tensor.transpose(
                                ptr[:], qr[:, h * 128:(h + 1) * 128], ident_sb[:]
                            )
                            nc.scalar.copy(
                                qT_sb[:, h, tt * 128:(tt + 1) * 128], ptr[:]
                            )
                        ptr = pptr.tile([128, 128], f32r, tag="tr", name="ptr")
                        nc.tensor.transpose(ptr[:], kr[:], ident_sb[:])
                        nc.scalar.copy(kT_sb[:, tt * 128:(tt + 1) * 128], ptr[:])
                    pending_tr = this_tr
                    scope.__exit__(None, None, None)
                for tt, qr, kr in pending_tr:
                    for h in range(G):
                        ptr = pptr.tile([128, 128], f32r, tag="tr", name="ptr")
                        nc.tensor.transpose(
                            ptr[:], qr[:, h * 128:(h + 1) * 128], ident_sb[:]
                        )
                        nc.scalar.copy(
                            qT_sb[:, h, tt * 128:(tt + 1) * 128], ptr[:]
                        )
                    ptr = pptr.tile([128, 128], f32r, tag="tr", name="ptr")
                    nc.tensor.transpose(ptr[:], kr[:], ident_sb[:])
                    nc.scalar.copy(kT_sb[:, tt * 128:(tt + 1) * 128], ptr[:])

            # ================= Phase 2+3: attention + output projection ======
            with (
                tc.tile_pool(name="wop", bufs=1) as wop,
                tc.tile_pool(name="expt", bufs=2) as expp,
                tc.tile_pool(name="denb", bufs=2) as denp,
                tc.tile_pool(name="bcb", bufs=2) as bcp,
                tc.tile_pool(name="pssc", bufs=3, space="PSUM") as pssc,
                tc.tile_pool(name="psden", bufs=1, space="PSUM") as psden,
                tc.tile_pool(name="psav", bufs=2, space="PSUM") as psav,
                tc.tile_pool(name="pswo", bufs=2, space="PSUM") as pswo,
            ):
                wo_sb = wop.tile([128, G, C], f32r)
                nc.sync.dma_start(
                    wo_sb[:], wo[:].rearrange("(h p) m -> p h m", p=128)
                )

                for tc_i in range(NTC):
                    scope = nc.named_scope(f"attn_tc{tc_i}")
                    scope.__enter__()
                    t0 = tc_i * 512
                    n_s = 4 * (tc_i + 1)
                    offs = [128 * (si - 4 * tc_i) if si >= 4 * tc_i else 0
                            for si in range(n_s)]
                    order = [4 * tc_i] + [si for si in range(n_s)
                                          if si != 4 * tc_i]
                    expTs = {}

                    def emit_scores(h):
                        expT = expp.tile([128, TT, 512], f32r, tag="expT",
                                         name="expT")
                        expTs[h] = expT
                        for si in range(n_s):
                            off = offs[si]
                            ps = pssc.tile([128, 512], f32, tag="sc", name="ps")
                            nc.tensor.matmul(
                                ps[:, off:512],
                                kT_sb[:, si * 128:(si + 1) * 128],
                                qT_sb[:, h, t0 + off:t0 + 512],
                                start=True, stop=True,
                            )
                            if si >= 4 * tc_i:
                                nc.vector.tensor_add(
                                    ps[:, off:off + 128],
                                    ps[:, off:off + 128], tri_sb[:],
                                )
                            nc.scalar.activation(
                                expT[:, si, off:512], ps[:, off:512],
                                mybir.ActivationFunctionType.Exp, scale=SCALE,
                            )

                    def emit_da(h):
                        expT = expTs.pop(h)
                        psd = psden.tile([1, 512], f32, tag="den", name="psd")
                        for i, si in enumerate(order):
                            off = offs[si]
                            nc.tensor.matmul(
                                psd[:, off:512], ones_sb[:],
                                expT[:, si, off:512],
                                start=(i == 0), stop=(i == n_s - 1),
                            )
                        den_r = denp.tile([1, 512], f32, tag="denr",
                                          name="den_r")
                        nc.vector.reciprocal_approx_fast(den_r[:], psd[:])
                        bc = bcp.tile([128, 512], f32, tag="bc", name="bc")
                        nc.gpsimd.partition_broadcast(bc[:], den_r[:])
                        pso = psav.tile([128, 512], f32, tag="av", name="pso")
                        for i, si in enumerate(order):
                            off = offs[si]
                            nc.tensor.matmul(
                                pso[:, off:512], v_sb[:, si, :],
                                expT[:, si, off:512],
                                start=(i == 0), stop=(i == n_s - 1),
                            )
                        nc.vector.tensor_mul(
                            outT_sb[:, h, t0:t0 + 512], pso[:], bc[:]
                        )

                    # 1-deep head pipelining: sc0 sc1 da0 sc2 da1 sc3 da2 da3
                    emit_scores(0)
                    emit_scores(1)
                    emit_da(0)
                    emit_scores(2)
                    emit_da(1)
                    emit_scores(3)
                    emit_da(2)
                    emit_da(3)
                    scope.__exit__(None, None, None)

                    scope = nc.named_scope(f"wo_tc{tc_i}")
                    scope.__enter__()
                    for t2 in range(4):
                        gt = tc_i * 4 + t2
                        for cc in range(4):
                            psw = pswo.tile([128, 512], f32, tag="wo",
                                            name="psw")
                            for h in range(G):
                                nc.tensor.matmul(
                                    psw[:],
                                    outT_sb[:, h, gt * 128:(gt + 1) * 128],
                                    wo_sb[:, h, cc * 512:(cc + 1) * 512],
                                    start=(h == 0), stop=(h == G - 1),
                                )
                            osb = opool.tile([128, 512], f32, tag="osb",
                                             name="osb")
                            nc.scalar.copy(osb[:], psw[:])
                            nc.gpsimd.dma_start(
                                out[gt * 128:(gt + 1) * 128,
                                    cc * 512:(cc + 1) * 512],
                                osb[:],
                            )
                    scope.__exit__(None, None, None)

    nc.finalize()
    return nc


def _prep_host(x, freqs_cos, freqs_sin, wq, wk, wv, wo):
    """Build per-core input maps."""
    x = np.asarray(x, dtype=np.float32)
    freqs_cos = np.asarray(freqs_cos, dtype=np.float32)
    freqs_sin = np.asarray(freqs_sin, dtype=np.float32)
    wq = np.asarray(wq, dtype=np.float32)
    wk = np.asarray(wk, dtype=np.float32)
    wv = np.asarray(wv, dtype=np.float32)
    wo = np.asarray(wo, dtype=np.float32)

    perm = np.concatenate([np.arange(0, HD, 2), np.arange(1, HD, 2)])
    xTs = [np.ascontiguousarray(x[b].T) for b in range(B)]
    cident = np.eye(128, dtype=np.float32)
    cones = np.ones((128, 1), dtype=np.float32)
    ds, dt = np.meshgrid(np.arange(128), np.arange(128), indexing="ij")
    ctri = np.where(ds <= dt, 0.0, MASK_BIAS).astype(np.float32)

    in_maps = []
    for c in range(NCORES):
        b, kv = c // 4, c % 4
        cols = []
        for g in range(G):
            h = kv * G + g
            cols.append(wq[:, h * HD:(h + 1) * HD][:, perm])
        cols.append(wk[:, kv * HD:(kv + 1) * HD][:, perm])
        cols.append(wv[:, kv * HD:(kv + 1) * HD])
        wqkv_c = np.ascontiguousarray(np.concatenate(cols, axis=1))
        wo_c = np.ascontiguousarray(wo[kv * G * HD:(kv + 1) * G * HD, :])
        in_maps.append({
            "xT": xTs[b],
            "wqkv": wqkv_c,
            "wo": wo_c,
            "fcos": freqs_cos,
            "fsin": freqs_sin,
            "cident": cident,
            "cones": cones,
            "ctri": ctri,
        })
    return in_maps


def _install_ntff_hook_shim():
    """bass_utils trace=True needs antenv.axon_hooks, absent in this image.
    Provide it in sys.modules and register the ctypes NTFF hook."""
    import types

    if "antenv.axon_hooks" in sys.modules:
        return
    mod = types.ModuleType("antenv.axon_hooks")
    mod._hook = None
    mod.set_axon_ntff_profile_hook = lambda h: setattr(mod, "_hook", h)
    mod.get_axon_ntff_profile_hook = lambda: mod._hook
    sys.modules["antenv.axon_hooks"] = mod
    try:
        from trn_agent_boot.trn_boot import _ntff_profile_via_ctypes

        mod._hook = _ntff_profile_via_ctypes("/opt/axon/libaxon_pjrt.so")
    except Exception:
        pass


def kernel(x, freqs_cos, freqs_sin, wq, wk, wv, wo, trace=False):
    global LAST_RESULTS
    import os

    if os.environ.get("LDW_OPT"):
        _patch_ldw_opt()
    from concourse.bass_utils import run_bass_kernel_spmd

    if trace:
        _install_ntff_hook_shim()

    if "nc" not in _CACHE:
        _CACHE["nc"] = _build()
    nc = _CACHE["nc"]

    in_maps = _prep_host(x, freqs_cos, freqs_sin, wq, wk, wv, wo)
    res = run_bass_kernel_spmd(nc, in_maps, core_ids=list(range(NCORES)),
                               trace=trace)
    LAST_RESULTS = res
    out = np.zeros((B, T, C), dtype=np.float32)
    for c in range(NCORES):
        out[c // 4] += res.results[c]["out"]
    return out


# revision 11
# speedup vs baseline: 1.8816x; 1.0331x over previous
"""Trainium2 Bass kernel for GQA multi-head attention with RoPE.

Problem: B=2, T=2048, C=2048, 16 q-heads, 4 kv-heads, HD=128, causal, RoPE.

Sharding (8 cores): tensor-parallel over the 4 kv-head groups x data-parallel
over the 2 batch elements. Core c handles batch c//4, kv-group c%4 (4 q-heads).
Each core computes x @ wq/wk/wv for its head group, RoPE, causal attention,
and a partial output projection (rows of wo for its heads). The host sums the
4 partial outputs per batch element.

Numerics: all matmuls run in float32r (TF32-like, full PE speed at free-dim
>= 256). Softmax skips the max-subtraction (scores are bounded ~N(0,1) here),
with the causal mask applied as a -1e5 additive bias on diagonal blocks and
fully-masked tiles skipped entirely.
"""

import sys

sys.path.insert(0, "/opt/trn_rl_repo")

import numpy as np

B, T, C = 2, 2048, 2048
N_KV = 4
G = 4           # q heads per kv head
HD = 128
NCORES = 8
TT = T // 128   # 16 t-tiles
CT = C // 128   # 16 c-tiles
NTC = 4         # 512-wide t chunks
SCALE = float(1.0 / np.sqrt(HD))
MASK_BIAS = -1.0e5

_CACHE = {}
LAST_RESULTS = None


def _patch_ldw_opt():
    """Enable walrus's LDWEIGHTS dedup pass (off by default in this repo)."""
    import concourse.bass_utils as bu

    if getattr(bu, "_ldw_patched", False):
        return
    orig = bu.run_command

    def run_command2(argv, **kw):
        argv = ["--enable-ldw-opt=true" if a == "--enable-ldw-opt=false" else a
                for a in argv]
        return orig(argv, **kw)

    bu.run_command = run_command2
    bu._ldw_patched = True


def _build():
    import concourse.bass as bass
    import concourse.tile as tile
    from concourse import mybir, bacc

    def bcast_mid(ap2d, reps):
        """[128, N] AP -> [128, reps, N] with a stride-0 middle dim."""
        return bass.AP(tensor=ap2d.tensor, offset=ap2d.offset,
                       ap=[list(ap2d.ap[0]), [0, reps], list(ap2d.ap[1])])

    f32, f32r = mybir.dt.float32, mybir.dt.float32r

    nc = bacc.Bacc()
    xT = nc.dram_tensor("xT", [C, T], f32r, kind="ExternalInput")
    wqkv = nc.dram_tensor("wqkv", [C, 768], f32r, kind="ExternalInput")
    wo = nc.dram_tensor("wo", [G * HD, C], f32r, kind="ExternalInput")
    fcos = nc.dram_tensor("fcos", [T, 64], f32, kind="ExternalInput")
    fsin = nc.dram_tensor("fsin", [T, 64], f32, kind="ExternalInput")
    cident = nc.dram_tensor("cident", [128, 128], f32r, kind="ExternalInput")
    cones = nc.dram_tensor("cones", [128, 1], f32r, kind="ExternalInput")
    ctri = nc.dram_tensor("ctri", [128, 128], f32, kind="ExternalInput")
    out = nc.dram_tensor("out", [T, C], f32, kind="ExternalOutput")

    with tile.TileContext(nc) as tc:
        with (
            tc.tile_pool(name="consts", bufs=1) as cpool,
            tc.tile_pool(name="persist", bufs=1) as ppool,
            tc.tile_pool(name="outbuf", bufs=3) as opool,
        ):
            ident_sb = cpool.tile([128, 128], f32r)
            nc.gpsimd.dma_start(ident_sb[:], cident[:])
            ones_sb = cpool.tile([128, 1], f32r)
            nc.gpsimd.dma_start(ones_sb[:], cones[:])
            tri_sb = cpool.tile([128, 128], f32)
            nc.gpsimd.dma_start(tri_sb[:], ctri[:])

            # ---- persistent activations ----
            qT_sb = ppool.tile([128, G, T], f32r)      # [d, h, t]
            kT_sb = ppool.tile([128, T], f32r)         # [d, s]
            v_sb = ppool.tile([128, TT, HD], f32r)     # [s%128, s//128, d]
            outT_sb = ppool.tile([128, G, T], f32r)    # [d, h, t]

            # ================= Phase 1: QKV projection + RoPE + transpose ====
            with (
                tc.tile_pool(name="weights", bufs=1) as wpool,
                tc.tile_pool(name="freqs", bufs=1) as fpool,
                tc.tile_pool(name="xt", bufs=3) as xtp,
                tc.tile_pool(name="ropet", bufs=4) as rtp,
                tc.tile_pool(name="qr", bufs=3) as qrp,
                tc.tile_pool(name="kr", bufs=3) as krp,
                tc.tile_pool(name="ppq", bufs=3, space="PSUM") as ppq,
                tc.tile_pool(name="ppkv", bufs=3, space="PSUM") as ppkv,
                tc.tile_pool(name="pptr", bufs=2, space="PSUM") as pptr,
            ):
                wqkv_sb = wpool.tile([128, CT, 768], f32r)
                wqkv_r = wqkv[:].rearrange("(n p) m -> p n m", p=128)
                for cg in range(CT // 4):
                    nc.gpsimd.dma_start(
                        wqkv_sb[:, cg * 4:(cg + 1) * 4, :],
                        wqkv_r[:, cg * 4:(cg + 1) * 4, :],
                    )
                fcos_sb = fpool.tile([128, TT, 64], f32)
                fsin_sb = fpool.tile([128, TT, 64], f32)
                nc.gpsimd.dma_start(
                    fcos_sb[:], fcos[:].rearrange("(n p) j -> p n j", p=128)
                )
                nc.gpsimd.dma_start(
                    fsin_sb[:], fsin[:].rearrange("(n p) j -> p n j", p=128)
                )

                pending_tr = []
                for ch in range(T // 256):  # 8 chunks of 256 t
                    scope = nc.named_scope(f"p1_ch{ch}")
                    scope.__enter__()
                    psq = [ppq.tile([128, 512], f32, tag="psq", name="psq")
                           for _ in range(2)]
                    pskv = [ppkv.tile([128, 256], f32, tag="pskv", name="pskv")
                            for _ in range(2)]
                    for cg in range(CT // 4):
                        xt = xtp.tile([128, 4, 256], f32r, tag="xt")
                        nc.sync.dma_start(
                            xt[:],
                            xT[:].rearrange("(n p) t -> p n t", p=128)[
                                cg * 4 * 128:cg * 4 * 128 + 128, :, :
                            ] if False else
                            bass.AP(
                                tensor=xT[:].tensor,
                                offset=(cg * 4 * 128) * T + ch * 256,
                                ap=[[T, 128], [128 * T, 4], [1, 256]],
                            ),
                        )
                        for ci in range(4):
                            ct = cg * 4 + ci
                            for t2 in range(2):
                                lhsT = xt[:, ci, t2 * 128:(t2 + 1) * 128]
                                nc.tensor.matmul(
                                    psq[t2][:], lhsT, wqkv_sb[:, ct, 0:512],
                                    start=(ct == 0), stop=(ct == CT - 1),
                                )
                                nc.tensor.matmul(
                                    pskv[t2][:], lhsT, wqkv_sb[:, ct, 512:768],
                                    start=(ct == 0), stop=(ct == CT - 1),
                                )
                    # rope (batched over the 4 q heads) — emitted now (DVE)
                    this_tr = []
                    for t2 in range(2):
                        tt = ch * 2 + t2
                        qr = qrp.tile([128, 512], f32r, tag="qr")
                        kr = krp.tile([128, 128], f32r, tag="kr")
                        cosb = bcast_mid(fcos_sb[:, tt, :], 4)
                        sinb = bcast_mid(fsin_sb[:, tt, :], 4)
                        qsrc = psq[t2][:].rearrange(
                            "p (h two j) -> p h two j", h=4, two=2
                        )
                        qdst = qr[:].rearrange(
                            "p (h two j) -> p h two j", h=4, two=2
                        )
                        te4, to4 = qsrc[:, :, 0, :], qsrc[:, :, 1, :]
                        a1 = rtp.tile([128, 4, 64], f32, tag="rt")
                        a2 = rtp.tile([128, 4, 64], f32, tag="rt")
                        nc.vector.tensor_mul(a1[:], te4, cosb)
                        nc.vector.tensor_mul(a2[:], to4, sinb)
                        nc.vector.tensor_sub(qdst[:, :, 0, :], a1[:], a2[:])
                        b1 = rtp.tile([128, 4, 64], f32, tag="rt")
                        b2 = rtp.tile([128, 4, 64], f32, tag="rt")
                        nc.vector.tensor_mul(b1[:], te4, sinb)
                        nc.vector.tensor_mul(b2[:], to4, cosb)
                        nc.vector.tensor_add(qdst[:, :, 1, :], b1[:], b2[:])
                        # K rope
                        kte, kto = pskv[t2][:, 0:64], pskv[t2][:, 64:128]
                        cos1 = fcos_sb[:, tt, :]
                        sin1 = fsin_sb[:, tt, :]
                        c1 = rtp.tile([128, 64], f32, tag="rtk")
                        c2 = rtp.tile([128, 64], f32, tag="rtk")
                        nc.vector.tensor_mul(c1[:], kte, cos1)
                        nc.vector.tensor_mul(c2[:], kto, sin1)
                        nc.vector.tensor_sub(kr[:, 0:64], c1[:], c2[:])
                        d1 = rtp.tile([128, 64], f32, tag="rtk")
                        d2 = rtp.tile([128, 64], f32, tag="rtk")
                        nc.vector.tensor_mul(d1[:], kte, sin1)
                        nc.vector.tensor_mul(d2[:], kto, cos1)
                        nc.vector.tensor_add(kr[:, 64:128], d1[:], d2[:])
                        nc.scalar.copy(v_sb[:, tt, :], pskv[t2][:, 128:256])
                        this_tr.append((tt, qr, kr))
                    # transposes for the PREVIOUS chunk (PE stays dense)
                    for tt, qr, kr in pending_tr:
                        for h in range(G):
                            ptr = pptr.tile([128, 128], f32r, tag="tr",
                                            name="ptr")
                            nc.tensor.transpose(
                                ptr[:], qr[:, h * 128:(h + 1) * 128], ident_sb[:]
                            )
                            nc.scalar.copy(
                                qT_sb[:, h, tt * 128:(tt + 1) * 128], ptr[:]
                            )
                        ptr = pptr.tile([128, 128], f32r, tag="tr", name="ptr")
                        nc.tensor.transpose(ptr[:], kr[:], ident_sb[:])
                        nc.scalar.copy(kT_sb[:, tt * 128:(tt + 1) * 128], ptr[:])
                    pending_tr = this_tr
                    scope.__exit__(None, None, None)
                for tt, qr, kr in pending_tr:
                    for h in range(G):
                        ptr = pptr.tile([128, 128], f32r, tag="tr", name="ptr")
                        nc.tensor.transpose(
                            ptr[:], qr[:, h * 128:(h + 1) * 128], ident_sb[:]
                        )
                        nc.scalar.copy(
                            qT_sb[:, h, tt * 128:(tt + 1) * 128], ptr[:]
                        )
                    ptr = pptr.tile([128, 128], f32r, tag="tr", name="ptr")
                    nc.tensor.transpose(ptr[:], kr[:], ident_sb[:])
                    nc.scalar.copy(kT_sb[:, tt * 128:(tt + 1) * 128], ptr[:])

            # ================= Phase 2+3: attention + output projection ======
            with (
                tc.tile_pool(name="wop", bufs=1) as wop,
                tc.tile_pool(name="expt", bufs=2) as expp,
                tc.tile_pool(name="denb", bufs=2) as denp,
                tc.tile_pool(name="bcb", bufs=2) as bcp,
                tc.tile_pool(name="pssc", bufs=3, space="PSUM") as pssc,
                tc.tile_pool(name="psden", bufs=1, space="PSUM") as psden,
                tc.tile_pool(name="psav", bufs=2, space="PSUM") as psav,
                tc.tile_pool(name="pswo", bufs=2, space="PSUM") as pswo,
            ):
                wo_sb = wop.tile([128, G, C], f32r)
                nc.sync.dma_start(
                    wo_sb[:], wo[:].rearrange("(h p) m -> p h m", p=128)
                )

                for tc_i in range(NTC):
                    scope = nc.named_scope(f"attn_tc{tc_i}")
                    scope.__enter__()
                    t0 = tc_i * 512
                    n_s = 4 * (tc_i + 1)
                    offs = [128 * (si - 4 * tc_i) if si >= 4 * tc_i else 0
                            for si in range(n_s)]
                    order = [4 * tc_i] + [si for si in range(n_s)
                                          if si != 4 * tc_i]
                    expTs = {}

                    def emit_scores(h):
                        expT = expp.tile([128, TT, 512], f32r, tag="expT",
                                         name="expT")
                        expTs[h] = expT
                        for si in range(n_s):
                            off = offs[si]
                            ps = pssc.tile([128, 512], f32, tag="sc", name="ps")
                            nc.tensor.matmul(
                                ps[:, off:512],
                                kT_sb[:, si * 128:(si + 1) * 128],
                                qT_sb[:, h, t0 + off:t0 + 512],
                                start=True, stop=True,
                            )
                            if si >= 4 * tc_i:
                                nc.vector.tensor_add(
                                    ps[:, off:off + 128],
                                    ps[:, off:off + 128], tri_sb[:],
                                )
                            nc.scalar.activation(
                                expT[:, si, off:512], ps[:, off:512],
                                mybir.ActivationFunctionType.Exp, scale=SCALE,
                            )

                    def emit_da(h):
                        expT = expTs.pop(h)
                        psd = psden.tile([1, 512], f32, tag="den", name="psd")
                        for i, si in enumerate(order):
                            off = offs[si]
                            nc.tensor.matmul(
                                psd[:, off:512], ones_sb[:],
                                expT[:, si, off:512],
                                start=(i == 0), stop=(i == n_s - 1),
                            )
                        den_r = denp.tile([1, 512], f32, tag="denr",
                                          name="den_r")
                        nc.vector.reciprocal_approx_fast(den_r[:], psd[:])
                        bc = bcp.tile([128, 512], f32, tag="bc", name="bc")
                        nc.gpsimd.partition_broadcast(bc[:], den_r[:])
                        pso = psav.tile([128, 512], f32, tag="av", name="pso")
                        for i, si in enumerate(order):
                            off = offs[si]
                            nc.tensor.matmul(
                                pso[:, off:512], v_sb[:, si, :],
                                expT[:, si, off:512],
                                start=(i == 0), stop=(i == n_s - 1),
                            )
                        nc.vector.tensor_mul(
                            outT_sb[:, h, t0:t0 + 512], pso[:], bc[:]
                        )

                    # 1-deep head pipelining: sc0 sc1 da0 sc2 da1 sc3 da2 da3
                    emit_scores(0)
                    emit_scores(1)
                    emit_da(0)
                    emit_scores(2)
                    emit_da(1)
                    emit_scores(3)
                    emit_da(2)
                    emit_da(3)
                    scope.__exit__(None, None, None)

                    scope = nc.named_scope(f"wo_tc{tc_i}")
                    scope.__enter__()
                    for t2 in range(4):
                        gt = tc_i * 4 + t2
                        for cc in range(4):
                            psw = pswo.tile([128, 512], f32, tag="wo",
                                            name="psw")
                            for h in range(G):
                                nc.tensor.matmul(
                                    psw[:],
                                    outT_sb[:, h, gt * 128:(gt + 1) * 128],
                                    wo_sb[:, h, cc * 512:(cc + 1) * 512],
                                    start=(h == 0), stop=(h == G - 1),
                                )
                            osb = opool.tile([128, 512], f32, tag="osb",
                                             name="osb")
                            nc.scalar.copy(osb[:], psw[:])
                            store_eng = nc.gpsimd if (t2 * 4 + cc) % 2 else nc.sync
                            store_eng.dma_start(
                                out[gt * 128:(gt + 1) * 128,
                                    cc * 512:(cc + 1) * 512],
                                osb[:],
                            )
                    scope.__exit__(None, None, None)

    nc.finalize()
    return nc


def _prep_host(x, freqs_cos, freqs_sin, wq, wk, wv, wo):
    """Build per-core input maps."""
    x = np.asarray(x, dtype=np.float32)
    freqs_cos = np.asarray(freqs_cos, dtype=np.float32)
    freqs_sin = np.asarray(freqs_sin, dtype=np.float32)
    wq = np.asarray(wq, dtype=np.float32)
    wk = np.asarray(wk, dtype=np.float32)
    wv = np.asarray(wv, dtype=np.float32)
    wo = np.asarray(wo, dtype=np.float32)

    perm = np.concatenate([np.arange(0, HD, 2), np.arange(1, HD, 2)])
    xTs = [np.ascontiguousarray(x[b].T) for b in range(B)]
    cident = np.eye(128, dtype=np.float32)
    cones = np.ones((128, 1), dtype=np.float32)
    ds, dt = np.meshgrid(np.arange(128), np.arange(128), indexing="ij")
    ctri = np.where(ds <= dt, 0.0, MASK_BIAS).astype(np.float32)

    in_maps = []
    for c in range(NCORES):
        b, kv = c // 4, c % 4
        cols = []
        for g in range(G):
            h = kv * G + g
            cols.append(wq[:, h * HD:(h + 1) * HD][:, perm])
        cols.append(wk[:, kv * HD:(kv + 1) * HD][:, perm])
        cols.append(wv[:, kv * HD:(kv + 1) * HD])
        wqkv_c = np.ascontiguousarray(np.concatenate(cols, axis=1))
        wo_c = np.ascontiguousarray(wo[kv * G * HD:(kv + 1) * G * HD, :])
        in_maps.append({
            "xT": xTs[b],
            "wqkv": wqkv_c,
            "wo": wo_c,
            "fcos": freqs_cos,
            "fsin": freqs_sin,
            "cident": cident,
            "cones": cones,
            "ctri": ctri,
        })
    return in_maps


def _install_ntff_hook_shim():
    """bass_utils trace=True needs antenv.axon_hooks, absent in this image.
    Provide it in sys.modules and register the ctypes NTFF hook."""
    import types

    if "antenv.axon_hooks" in sys.modules:
        return
    mod = types.ModuleType("antenv.axon_hooks")
    mod._hook = None
    mod.set_axon_ntff_profile_hook = lambda h: setattr(mod, "_hook", h)
    mod.get_axon_ntff_profile_hook = lambda: mod._hook
    sys.modules["antenv.axon_hooks"] = mod
    try:
        from trn_agent_boot.trn_boot import _ntff_profile_via_ctypes

        mod._hook = _ntff_profile_via_ctypes("/opt/axon/libaxon_pjrt.so")
    except Exception:
        pass


def kernel(x, freqs_cos, freqs_sin, wq, wk, wv, wo, trace=False):
    global LAST_RESULTS
    import os

    if os.environ.get("LDW_OPT"):
        _patch_ldw_opt()
    from concourse.bass_utils import run_bass_kernel_spmd

    if trace:
        _install_ntff_hook_shim()

    if "nc" not in _CACHE:
        _CACHE["nc"] = _build()
    nc = _CACHE["nc"]

    in_maps = _prep_host(x, freqs_cos, freqs_sin, wq, wk, wv, wo)
    res = run_bass_kernel_spmd(nc, in_maps, core_ids=list(range(NCORES)),
                               trace=trace)
    LAST_RESULTS = res
    out = np.zeros((B, T, C), dtype=np.float32)
    for c in range(NCORES):
        out[c // 4] += res.results[c]["out"]
    return out
